# revision 15
# baseline (speedup 1.0000x reference)
"""Causal self-attention (B=2, T=2048, D=1024, H=16) on 8 TRN2 NeuronCores.

Sharding: data parallel on batch (2 groups of 4 cores) x tensor parallel on
heads (4 heads per core, splitting the qkv / out projections). Each core
computes a partial output ``X[b] -> partial_out`` for its 4 heads; the host
sums the 4 partials per batch. No device collectives.

v2 vs v1:
  - all DRAM traffic bf16 (inputs, weights, output partials): halves the
    serial input-DMA prologue and the output tail.
  - q/k stored as fp8(e4m3) in a DoubleRow-folded layout (head h on
    partitions h*32..h*32+32, d split over a free axis of 2), so the
    QK^T score matmuls run in DoubleRow perf mode at 2 rows/cycle.
    The 1/8 softmax scale is folded into Wq on the host before the bf16
    cast; fp8 rounding of q/k costs ~1.1e-2 rel err (validated on host).
  - att probabilities / v / zTn / projections in bf16 (fp8 AV fails the
    2e-2 budget), exp stays on ACT in f32->bf16.
  - exact causal trimming on diagonal k-tiles (bf16 has no >=256-wide
    matmul constraint, unlike fp32r).
  - software pipelining: qkv-projection chunks for block qi+1 and the
    out-projection of block qi-1 are interleaved between attention
    group-steps of block qi, filling the PE during the ACT-bound exp
    phases. Output copies run on the (otherwise idle) Pool engine.

Per-core math:
  qT8/kT8 [32h+dlo, dhi, t] = fp8(Wq/k_perm.T @ X[b].T)  (q pre-scaled 1/8)
  v      [t,d] = X[b] @ Wv_localT                  (bf16)
  scoresT[k,q] = kT8.T @ qT8                       (fp8 DoubleRow, d=64)
  attT   [k,q] = bf16(exp(scoresT)) * causal mask
  zT     [d,q] = v_ext.T @ attT                    (bf16; ones col -> denom)
  out    [t,e] = bf16((zT/denom).T @ Wout_localT)  (host sums partials)
"""

import sys

sys.path.insert(0, "/opt/trn_rl_repo")

import numpy as np

import concourse.bacc as bacc
import concourse.mybir as mybir
import concourse.tile as tile

B, T, D, H = 2, 2048, 1024, 16
HD = D // H              # 64
NCORES = 8
NGROUP = 4               # cores per batch (tensor-parallel group)
HPC = H // NGROUP        # heads per core = 4
DLOC = HPC * HD          # local model dims per core = 256
QB = 512                 # q-block (matmul moving free dim)
NQB = T // QB            # 4
KT = 128                 # k-tile (psum partitions)
NKT = T // KT            # 16
NDT = D // 128           # 8 d-tiles

F32 = mybir.dt.float32
BF16 = mybir.dt.bfloat16
F8 = mybir.dt.float8e4
DR = mybir.MatmulPerfMode.DoubleRow


def build_program(reps=1):
    nc = bacc.Bacc("TRN2", target_bir_lowering=False, debug=False,
                   num_devices=NCORES)

    xt_d = nc.declare_dram_parameter("xt", [D, T], BF16, isOutput=False)
    wqkv_d = nc.declare_dram_parameter("wqkv", [D, 3 * DLOC], BF16, isOutput=False)
    wout_d = nc.declare_dram_parameter("wout", [DLOC, D], BF16, isOutput=False)
    mask_d = nc.declare_dram_parameter("masks", [KT, 128], BF16, isOutput=False)
    ones_d = nc.declare_dram_parameter("ones", [128, NKT, HPC, 1], BF16, isOutput=False)
    out_d = nc.declare_dram_parameter("out", [T, D], BF16, isOutput=True)

    with tile.TileContext(nc) as tc:
        with (
            tc.tile_pool(name="cst", bufs=1) as cst,
            tc.tile_pool(name="att", bufs=4) as attp,
            tc.tile_pool(name="sm", bufs=2) as smp,
            tc.tile_pool(name="zn", bufs=4) as znp,
            tc.tile_pool(name="ops", bufs=4) as opsp,
            tc.tile_pool(name="ps", bufs=3, space="PSUM") as ps,    # [128,2,512]
            tc.tile_pool(name="zps", bufs=2, space="PSUM") as zps,  # [128,512]
        ):
            xt = cst.tile([128, NDT, T], BF16, tag="xt")
            wqkv = cst.tile([128, NDT, 3 * DLOC], BF16, tag="wqkv")
            wout = cst.tile([128, DLOC // 128, D], BF16, tag="wout")
            masks = cst.tile([128, 128], BF16, tag="masks")
            # fp8 q/k in DoubleRow-folded layout: partition = h*32 + (d%32),
            # free axis 0 = d//32, free axis 1 = t. Matmul APs can only
            # base at partition 0/32/64, so head 3 (partitions 96:128) is
            # DMA-shuffled into a separate base-0 tile after each proj.
            qT8 = cst.tile([128, 2, T], F8, tag="qT8")
            kT8 = cst.tile([128, 2, T], F8, tag="kT8")
            qT8b = cst.tile([32, 2, T], F8, tag="qT8b")
            kT8b = cst.tile([32, 2, T], F8, tag="kT8b")

            def qk_ap(h, src, srcb):
                return srcb if h == 3 else src[h * 32:(h + 1) * 32]
            vext = cst.tile([128, NKT, HPC, HD + 1], BF16, tag="vext")

            def projq_chunks(tb):
                """qk projection for q-block tb: 2 chunks (q pair, k pair)."""
                def chunk(pair):
                    pt = ps.tile([128, 2, QB], F32, tag="ps")
                    for jj in range(2):
                        ecol = (pair * 2 + jj) * 128
                        for dt_ in range(NDT):
                            nc.tensor.matmul(
                                pt[:, jj, :],
                                wqkv[:, dt_, ecol:ecol + 128],
                                xt[:, dt_, tb * QB:(tb + 1) * QB],
                                start=(dt_ == 0), stop=(dt_ == NDT - 1),
                            )
                    dst, dstb = (qT8, qT8b) if pair == 0 else (kT8, kT8b)
                    # DVE copy does the f32 -> fp8e4 cast (keeps ACT free
                    # for the exp stream)
                    nc.vector.tensor_copy(dst[:, :, tb * QB:(tb + 1) * QB], pt[:])
                    nc.sync.dma_start(dstb[:, :, tb * QB:(tb + 1) * QB],
                                      dst[96:128, :, tb * QB:(tb + 1) * QB])
                return [lambda pair=pair: chunk(pair) for pair in range(2)]

            def projv_chunks(tb):
                """v projection for q-block tb: 2 chunks (one per t-128-pair)."""
                def chunk(tp):
                    pt = ps.tile([128, 2, QB], F32, tag="ps")
                    for j in range(2):
                        tch = tp * 2 + j
                        for dt_ in range(NDT):
                            nc.tensor.matmul(
                                pt[:, j, 0:DLOC],
                                xt[:, dt_, tch * 128:(tch + 1) * 128],
                                wqkv[:, dt_, 2 * DLOC:3 * DLOC],
                                start=(dt_ == 0), stop=(dt_ == NDT - 1),
                            )
                        nc.vector.tensor_copy(
                            vext[:, tch, :, 0:HD],
                            pt[:, j, 0:DLOC].rearrange("p (h d) -> p h d", h=HPC),
                        )
                return [lambda tp=tp: chunk(tp) for tp in (2 * tb, 2 * tb + 1)]

            def oproj_chunks(qo, zo, split_copy=False):
                """output projection of q-block qo: 4 chunks (one per 128 rows)."""
                def chunk(tch):
                    po = ps.tile([128, 2, QB], F32, tag="ps")
                    for et in range(2):
                        for dt_ in range(2):
                            nc.tensor.matmul(
                                po[:, et, :],
                                zo[:, dt_, tch * 128:(tch + 1) * 128],
                                wout[:, dt_, et * QB:(et + 1) * QB],
                                start=(dt_ == 0), stop=(dt_ == 1),
                            )
                    ot = opsp.tile([128, 2, QB], BF16, tag="ot")
                    if split_copy:
                        # epilogue: ACT is idle, halve the drain latency
                        nc.vector.tensor_copy(ot[:, 0, :], po[:, 0, :])
                        nc.scalar.copy(ot[:, 1, :], po[:, 1, :])
                    else:
                        nc.vector.tensor_copy(ot[:], po[:])
                    row = qo * QB + tch * 128
                    nc.sync.dma_start(
                        out_d[row:row + 128, :],
                        ot.rearrange("p a q -> p (a q)"))
                return [lambda tch=tch: chunk(tch) for tch in range(QB // 128)]

            def load_inputs():
                wqkv_r = wqkv_d.rearrange("(a p) e -> p a e", p=128)
                xt_r = xt_d.rearrange("(a p) t -> p a t", p=128)
                # interleave per d-tile so the first qk-proj matmul starts
                # after ~0.25MB of DMA
                nc.sync.dma_start(wqkv[:, 0:1, 0:2 * DLOC], wqkv_r[:, 0:1, 0:2 * DLOC])
                nc.sync.dma_start(xt[:, 0:1, 0:QB], xt_r[:, 0:1, 0:QB])
                nc.sync.dma_start(wqkv[:, 1:2, 0:2 * DLOC], wqkv_r[:, 1:2, 0:2 * DLOC])
                nc.sync.dma_start(xt[:, 1:2, 0:QB], xt_r[:, 1:2, 0:QB])
                for dt_ in range(2, NDT, 2):
                    nc.sync.dma_start(wqkv[:, dt_:dt_ + 2, 0:2 * DLOC],
                                      wqkv_r[:, dt_:dt_ + 2, 0:2 * DLOC])
                    nc.sync.dma_start(xt[:, dt_:dt_ + 2, 0:QB],
                                      xt_r[:, dt_:dt_ + 2, 0:QB])
                for dt_ in range(0, NDT, 2):   # v weight columns
                    nc.sync.dma_start(wqkv[:, dt_:dt_ + 2, 2 * DLOC:3 * DLOC],
                                      wqkv_r[:, dt_:dt_ + 2, 2 * DLOC:3 * DLOC])
                for tchunk in range(1, 4):
                    sl = slice(tchunk * QB, (tchunk + 1) * QB)
                    nc.sync.dma_start(xt[:, :, sl], xt_r[:, :, sl])
                nc.sync.dma_start(wout[:], wout_d.rearrange("(a p) e -> p a e", p=128))
                nc.sync.dma_start(masks[:], mask_d[:])
                nc.sync.dma_start(vext[:, :, :, HD:HD + 1], ones_d[:])

            def attention(qi, fill):
                """Attention for q-block qi; pops deferred PE work from
                `fill` between group-steps to occupy the PE during exp."""
                zTn = znp.tile([128, 2, QB], BF16, tag="zTn")
                G = 2 * (qi + 1)          # k-groups of 2 k-tiles
                n_steps = 2 * (G + 1)
                step = 0
                emitted = 0

                def fill_maybe():
                    nonlocal emitted
                    want = len(fill) * step // n_steps
                    while emitted < want:
                        fill[emitted]()
                        emitted += 1

                # relative masked-strip offset of k-tile (g, j); None if
                # the tile is entirely below the diagonal (no masking)
                def _r(gg, j):
                    if gg < G - 2:
                        return None
                    return (gg - (G - 2)) * 256 + j * 128

                def _lo(r):
                    return 0 if r is None else r

                for p in range(2):        # head pairs (0,1) then (2,3)
                    zts = {}
                    att_tiles = {}
                    with nc.named_scope(f"att_q{qi}_p{p}"):
                        for g in range(G + 1):
                            for h in (2 * p, 2 * p + 1):
                                off, hv = (h % 2) * 64, h // 2
                                if g < G:
                                    if g == 0:
                                        zts[h] = zps.tile([128, QB], F32, tag="zt",
                                                          name=f"zt{h}")
                                    sc = ps.tile([128, 2, QB], F32, tag="ps")
                                    for j in range(2):
                                        kt_i = g * 2 + j
                                        lo = _lo(_r(g, j))
                                        nc.tensor.matmul(
                                            sc[:, j, lo:],
                                            qk_ap(h, kT8, kT8b)[
                                                :, :, kt_i * 128:(kt_i + 1) * 128],
                                            qk_ap(h, qT8, qT8b)[
                                                :, :, qi * QB + lo:(qi + 1) * QB],
                                            start=True, stop=True,
                                            perf_mode=DR,
                                        )
                                    at = attp.tile([128, 2, QB], BF16, tag="at")
                                    lo01 = _lo(_r(g, 0))
                                    nc.scalar.activation(
                                        at[:, :, lo01:], sc[:, :, lo01:],
                                        mybir.ActivationFunctionType.Exp)
                                    for j in range(2):
                                        r = _r(g, j)
                                        if r is None:
                                            continue
                                        nc.vector.tensor_mul(
                                            at[:, j, r:r + 128],
                                            at[:, j, r:r + 128],
                                            masks[:, 0:128])
                                    att_tiles[h, g] = at
                                if g >= 1:
                                    ap = att_tiles.pop((h, g - 1))
                                    for j in range(2):
                                        kt_i = (g - 1) * 2 + j
                                        lo = _lo(_r(g - 1, j))
                                        nc.tensor.matmul(
                                            zts[h][0:HD + 1, lo:],
                                            vext[:, kt_i, h, :],
                                            ap[:, j, lo:],
                                            start=(g - 1 == 0 and j == 0),
                                            stop=(g - 1 == G - 1 and j == 1),
                                        )
                                if g == G:
                                    zt = zts[h]
                                    scr = smp.tile([128, QB], F32, tag="scr")
                                    bc = smp.tile([128, QB], F32, tag="bc")
                                    nc.vector.reciprocal(scr[0:1, :], zt[HD:HD + 1, :])
                                    nc.gpsimd.partition_broadcast(
                                        bc[:], scr[0:1, :], channels=128)
                                    nc.vector.tensor_mul(
                                        zTn[off:off + 64, hv, :],
                                        zt[0:HD, :], bc[off:off + 64, :])
                            step += 1
                            fill_maybe()
                while emitted < len(fill):
                    fill[emitted]()
                    emitted += 1
                return zTn

            def body(_i):
                load_inputs()
                with nc.named_scope("proj_q0"):
                    for c in projq_chunks(0) + projv_chunks(0):
                        c()
                # fill schedule: ACT-vs-PE deficit grows with the block index
                # (2.6us * (qi+1)), so defer the out-projections as late as
                # zTn buffering allows.
                zTn_prev = {}
                for qi in range(NQB):
                    fill = []
                    if qi + 1 < NQB:
                        with nc.named_scope(f"defer_proj_q{qi + 1}"):
                            fill += projq_chunks(qi + 1) + projv_chunks(qi + 1)
                    if qi == 3:
                        for qo in (0, 1, 2):
                            fill += oproj_chunks(qo, zTn_prev.pop(qo))
                    zTn_prev[qi] = attention(qi, fill)
                with nc.named_scope("oproj_q3"):
                    for c in oproj_chunks(NQB - 1, zTn_prev.pop(NQB - 1),
                                          split_copy=True):
                        c()

            if reps == 1:
                body(0)
            else:
                with tc.For_i(0, reps, 1, staggered_reset=True,
                              hint_engines=(mybir.EngineType.PE,)) as i:
                    body(i)

    nc.compile()
    return nc


def make_in_maps(X, W_qkv, W_out):
    """Host-side sharding: per-core input dict (bf16 DRAM traffic)."""
    import ml_dtypes
    bf16 = ml_dtypes.bfloat16

    X = np.asarray(X, dtype=np.float32)
    W_qkv = np.asarray(W_qkv, dtype=np.float32)
    W_out = np.asarray(W_out, dtype=np.float32)

    kp = np.arange(KT)[:, None]
    qf = np.arange(128)[None, :]
    masks = (qf >= kp).astype(bf16)

    # DoubleRow fold: PSUM partition p of jj-block holds local column
    # (p//32)*64 + jj*32 + (p%32)
    perm = np.array([(p // 32) * 64 + jj * 32 + (p % 32)
                     for jj in range(2) for p in range(128)])

    in_maps = []
    for c in range(NCORES):
        b, hg = divmod(c, NGROUP)
        rows = slice(hg * DLOC, (hg + 1) * DLOC)
        wq = (W_qkv[0 * D:1 * D][rows] * 0.125)[perm].T  # fold 1/sqrt(hd) into q
        wk = W_qkv[1 * D:2 * D][rows][perm].T
        wv = W_qkv[2 * D:3 * D][rows].T
        in_maps.append({
            "xt": np.ascontiguousarray(X[b].T).astype(bf16),
            "wqkv": np.ascontiguousarray(
                np.concatenate([wq, wk, wv], axis=1)).astype(bf16),
            "wout": np.ascontiguousarray(W_out[:, rows].T).astype(bf16),
            "masks": masks,
            "ones": np.ones((128, NKT, HPC, 1), dtype=bf16),
        })
    return in_maps


def combine_outputs(results):
    """Sum the 4 tensor-parallel partials per batch -> [B, T, D]."""
    out = np.zeros((B, T, D), dtype=np.float32)
    for c, r in enumerate(results):
        out[c // NGROUP] += np.asarray(r["out"], dtype=np.float32)
    return out


_cached = {}


def kernel(X, W_qkv, W_out):
    from concourse.bass_utils import run_bass_kernel_spmd

    if "nc" not in _cached:
        _cached["nc"] = build_program(reps=1)
    nc = _cached["nc"]
    in_maps = make_in_maps(X, W_qkv, W_out)
    r = run_bass_kernel_spmd(nc, in_maps, core_ids=list(range(NCORES)))
    return combine_outputs(r.results)


# revision 38
# speedup vs baseline: 1.0551x; 1.0551x over previous
"""Causal self-attention (B=2, T=2048, D=1024, H=16) on 8 TRN2 NeuronCores.

Sharding: data parallel on batch (2 groups of 4 cores) x tensor parallel on
heads (4 heads per core, splitting the qkv / out projections). Each core
computes a partial output ``X[b] -> partial_out`` for its 4 heads; the host
sums the 4 partials per batch. No device collectives.

v2 vs v1:
  - all DRAM traffic bf16 (inputs, weights, output partials): halves the
    serial input-DMA prologue and the output tail.
  - q/k stored as fp8(e4m3) in a DoubleRow-folded layout (head h on
    partitions h*32..h*32+32, d split over a free axis of 2), so the
    QK^T score matmuls run in DoubleRow perf mode at 2 rows/cycle.
    The 1/8 softmax scale is folded into Wq on the host before the bf16
    cast; fp8 rounding of q/k costs ~1.1e-2 rel err (validated on host).
  - att probabilities / v / zTn / projections in bf16 (fp8 AV fails the
    2e-2 budget), exp stays on ACT in f32->bf16.
  - exact causal trimming on diagonal k-tiles (bf16 has no >=256-wide
    matmul constraint, unlike fp32r).
  - software pipelining: qkv-projection chunks for block qi+1 and the
    out-projection of block qi-1 are interleaved between attention
    group-steps of block qi, filling the PE during the ACT-bound exp
    phases. Output copies run on the (otherwise idle) Pool engine.

Per-core math:
  qT8/kT8 [32h+dlo, dhi, t] = fp8(Wq/k_perm.T @ X[b].T)  (q pre-scaled 1/8)
  v      [t,d] = X[b] @ Wv_localT                  (bf16)
  scoresT[k,q] = kT8.T @ qT8                       (fp8 DoubleRow, d=64)
  attT   [k,q] = bf16(exp(scoresT)) * causal mask
  zT     [d,q] = v_ext.T @ attT                    (bf16; ones col -> denom)
  out    [t,e] = bf16((zT/denom).T @ Wout_localT)  (host sums partials)
"""

import sys

sys.path.insert(0, "/opt/trn_rl_repo")

import numpy as np

import concourse.bacc as bacc
import concourse.mybir as mybir
import concourse.tile as tile

B, T, D, H = 2, 2048, 1024, 16
HD = D // H              # 64
NCORES = 8
NGROUP = 4               # cores per batch (tensor-parallel group)
HPC = H // NGROUP        # heads per core = 4
DLOC = HPC * HD          # local model dims per core = 256
QB = 512                 # q-block (matmul moving free dim)
NQB = T // QB            # 4
KT = 128                 # k-tile (psum partitions)
NKT = T // KT            # 16
NDT = D // 128           # 8 d-tiles

F32 = mybir.dt.float32
BF16 = mybir.dt.bfloat16
F8 = mybir.dt.float8e4
DR = mybir.MatmulPerfMode.DoubleRow

# "fp8" (DoubleRow QK^T) measured ~176us vs "bf16" ~157us on HW: DR's
# 256-col LDWEIGHTS can't hide behind the halved matmul, so it loses.
SC_MODE = "bf16"


def build_program(reps=1, sc_mode=None):
    sc_mode = sc_mode or SC_MODE
    nc = bacc.Bacc("TRN2", target_bir_lowering=False, debug=False,
                   num_devices=NCORES)

    xt_d = nc.declare_dram_parameter("xt", [D, T], BF16, isOutput=False)
    wqkv_d = nc.declare_dram_parameter("wqkv", [D, 3 * DLOC], BF16, isOutput=False)
    wout_d = nc.declare_dram_parameter("wout", [DLOC, D], BF16, isOutput=False)
    mask_d = nc.declare_dram_parameter("masks", [KT, 128], BF16, isOutput=False)
    out_d = nc.declare_dram_parameter("out", [T, D], BF16, isOutput=True)

    with tile.TileContext(nc) as tc:
        with (
            tc.tile_pool(name="cst", bufs=1) as cst,
            tc.tile_pool(name="att", bufs=4) as attp,
            tc.tile_pool(name="sm", bufs=2) as smp,
            tc.tile_pool(name="zn", bufs=4) as znp,
            tc.tile_pool(name="ops", bufs=4) as opsp,
            tc.tile_pool(name="ps", bufs=3, space="PSUM") as ps,    # [128,2,512]
            tc.tile_pool(name="zps", bufs=2, space="PSUM") as zps,  # [128,512]
        ):
            xt = cst.tile([128, NDT, T], BF16, tag="xt")
            wqkv = cst.tile([128, NDT, 3 * DLOC], BF16, tag="wqkv")
            wout = cst.tile([128, DLOC // 128, D], BF16, tag="wout")
            masks = cst.tile([128, 128], BF16, tag="masks")
            # fp8 q/k in DoubleRow-folded layout: partition = h*32 + (d%32),
            # free axis 0 = d//32, free axis 1 = t. Matmul APs can only
            # base at partition 0/32/64, so head 3 (partitions 96:128) is
            # DMA-shuffled into a separate base-0 tile after each proj.
            # bf16 mode: v1 layout, partition = (h%2)*64 + d, free axis 0
            # = h//2.
            qkdt = F8 if sc_mode == "fp8" else BF16
            qT8 = cst.tile([128, 2, T], qkdt, tag="qT8")
            kT8 = cst.tile([128, 2, T], qkdt, tag="kT8")
            if sc_mode == "fp8":
                qT8b = cst.tile([32, 2, T], F8, tag="qT8b")
                kT8b = cst.tile([32, 2, T], F8, tag="kT8b")

            def qk_ap(h, src, srcb):
                return srcb if h == 3 else src[h * 32:(h + 1) * 32]
            vext = cst.tile([128, NKT, HPC, HD + 1], BF16, tag="vext")

            def projq_chunks(tb):
                """qk projection for q-block tb: 2 chunks (q pair, k pair)."""
                def chunk(pair):
                    pt = ps.tile([128, 2, QB], F32, tag="ps")
                    for jj in range(2):
                        ecol = (pair * 2 + jj) * 128
                        for dt_ in range(NDT):
                            nc.tensor.matmul(
                                pt[:, jj, :],
                                wqkv[:, dt_, ecol:ecol + 128],
                                xt[:, dt_, tb * QB:(tb + 1) * QB],
                                start=(dt_ == 0), stop=(dt_ == NDT - 1),
                            )
                    dst = qT8 if pair == 0 else kT8
                    # DVE copy does the f32 -> fp8e4/bf16 cast (keeps ACT
                    # free for the exp stream)
                    nc.vector.tensor_copy(dst[:, :, tb * QB:(tb + 1) * QB], pt[:])
                    if sc_mode == "fp8":
                        dstb = qT8b if pair == 0 else kT8b
                        nc.sync.dma_start(dstb[:, :, tb * QB:(tb + 1) * QB],
                                          dst[96:128, :, tb * QB:(tb + 1) * QB])
                return [lambda pair=pair: chunk(pair) for pair in range(2)]

            def projv_chunks(tb):
                """v projection for q-block tb: 2 chunks (one per t-128-pair)."""
                def chunk(tp):
                    pt = ps.tile([128, 2, QB], F32, tag="ps")
                    for j in range(2):
                        tch = tp * 2 + j
                        for dt_ in range(NDT):
                            nc.tensor.matmul(
                                pt[:, j, 0:DLOC],
                                xt[:, dt_, tch * 128:(tch + 1) * 128],
                                wqkv[:, dt_, 2 * DLOC:3 * DLOC],
                                start=(dt_ == 0), stop=(dt_ == NDT - 1),
                            )
                        nc.vector.tensor_copy(
                            vext[:, tch, :, 0:HD],
                            pt[:, j, 0:DLOC].rearrange("p (h d) -> p h d", h=HPC),
                        )
                return [lambda tp=tp: chunk(tp) for tp in (2 * tb, 2 * tb + 1)]

            def oproj_chunks(qo, zo):
                """output projection of q-block qo: 4 chunks (one per 128 rows)."""
                def chunk(tch):
                    po = ps.tile([128, 2, QB], F32, tag="ps")
                    for et in range(2):
                        for dt_ in range(2):
                            nc.tensor.matmul(
                                po[:, et, :],
                                zo[:, dt_, tch * 128:(tch + 1) * 128],
                                wout[:, dt_, et * QB:(et + 1) * QB],
                                start=(dt_ == 0), stop=(dt_ == 1),
                            )
                    ot = opsp.tile([128, 2, QB], BF16, tag="ot")
                    nc.vector.tensor_copy(ot[:], po[:])
                    row = qo * QB + tch * 128
                    nc.sync.dma_start(
                        out_d[row:row + 128, :],
                        ot.rearrange("p a q -> p (a q)"))
                return [lambda tch=tch: chunk(tch) for tch in range(QB // 128)]

            def oproj_final(qo, zo):
                """Epilogue out-projection: drains split across DVE and ACT
                with per-half DMAs — minimizes the post-attention tail."""
                def chunk(tch):
                    po = ps.tile([128, 2, QB], F32, tag="ps")
                    for et in range(2):
                        for dt_ in range(2):
                            nc.tensor.matmul(
                                po[:, et, :],
                                zo[:, dt_, tch * 128:(tch + 1) * 128],
                                wout[:, dt_, et * QB:(et + 1) * QB],
                                start=(dt_ == 0), stop=(dt_ == 1),
                            )
                    ot = opsp.tile([128, 2, QB], BF16, tag="ot")
                    row = qo * QB + tch * 128
                    nc.vector.tensor_copy(ot[:, 0, :], po[:, 0, :])
                    nc.scalar.copy(ot[:, 1, :], po[:, 1, :])
                    if tch == QB // 128 - 1:
                        # last chunk: per-half DMAs shorten the critical tail
                        nc.sync.dma_start(out_d[row:row + 128, 0:QB],
                                          ot[:, 0, :])
                        nc.sync.dma_start(out_d[row:row + 128, QB:2 * QB],
                                          ot[:, 1, :])
                    else:
                        nc.sync.dma_start(out_d[row:row + 128, :],
                                          ot.rearrange("p a q -> p (a q)"))
                return [lambda tch=tch: chunk(tch)
                        for tch in range(QB // 128)]

            def load_inputs():
                # denominator ones-column: generated on-device (a strided
                # 128B DMA costs ~3.6us of descriptor time and blocked the
                # first AV matmuls head-of-line)
                nc.gpsimd.memset(vext[:, :, :, HD:HD + 1], 1.0)
                wqkv_r = wqkv_d.rearrange("(a p) e -> p a e", p=128)
                xt_r = xt_d.rearrange("(a p) t -> p a t", p=128)
                # interleave per d-tile so the first qk-proj matmul starts
                # after ~0.25MB of DMA
                nc.sync.dma_start(wqkv[:, 0:1, 0:2 * DLOC], wqkv_r[:, 0:1, 0:2 * DLOC])
                nc.sync.dma_start(xt[:, 0:1, 0:QB], xt_r[:, 0:1, 0:QB])
                nc.sync.dma_start(masks[:], mask_d[:])
                nc.sync.dma_start(wqkv[:, 1:2, 0:2 * DLOC], wqkv_r[:, 1:2, 0:2 * DLOC])
                nc.sync.dma_start(xt[:, 1:2, 0:QB], xt_r[:, 1:2, 0:QB])
                for dt_ in range(2, NDT, 2):
                    nc.sync.dma_start(wqkv[:, dt_:dt_ + 2, 0:2 * DLOC],
                                      wqkv_r[:, dt_:dt_ + 2, 0:2 * DLOC])
                    nc.sync.dma_start(xt[:, dt_:dt_ + 2, 0:QB],
                                      xt_r[:, dt_:dt_ + 2, 0:QB])
                for dt_ in range(0, NDT, 2):   # v weight columns
                    nc.sync.dma_start(wqkv[:, dt_:dt_ + 2, 2 * DLOC:3 * DLOC],
                                      wqkv_r[:, dt_:dt_ + 2, 2 * DLOC:3 * DLOC])
                for tchunk in range(1, 4):
                    sl = slice(tchunk * QB, (tchunk + 1) * QB)
                    nc.sync.dma_start(xt[:, :, sl], xt_r[:, :, sl])
                nc.sync.dma_start(wout[:], wout_d.rearrange("(a p) e -> p a e", p=128))

            def attention(qi, fill):
                """Attention for q-block qi; pops deferred PE work from
                `fill` between group-steps to occupy the PE during exp."""
                zTn = znp.tile([128, 2, QB], BF16, tag="zTn")
                G = 2 * (qi + 1)          # k-groups of 2 k-tiles
                n_steps = 4 * (G + 1)     # fill granularity: per (p, g, h)
                step = 0
                emitted = 0

                # hold chunks back past the loop so the PE has work under
                # the end-of-block exp backlog / normalization latency
                reserve = 3 if qi == NQB - 1 else 1
                reserve = min(reserve, max(len(fill) - 1, 0))

                def fill_maybe():
                    nonlocal emitted
                    want = min(len(fill) * step // n_steps,
                               len(fill) - reserve)
                    while emitted < want:
                        fill[emitted]()
                        emitted += 1

                # relative masked-strip offset of k-tile (g, j); None if
                # the tile is entirely below the diagonal (no masking)
                def _r(gg, j):
                    if gg < G - 2:
                        return None
                    return (gg - (G - 2)) * 256 + j * 128

                def _lo(r):
                    return 0 if r is None else r

                for p in range(2):        # head pairs (0,1) then (2,3)
                    zts = {}
                    att_tiles = {}
                    with nc.named_scope(f"att_q{qi}_p{p}"):
                        for g in range(G + 1):
                            for h in (2 * p, 2 * p + 1):
                                off, hv = (h % 2) * 64, h // 2
                                if g < G:
                                    if g == 0:
                                        zts[h] = zps.tile([128, QB], F32, tag="zt",
                                                          name=f"zt{h}")
                                    sc = ps.tile([128, 2, QB], F32, tag="ps")
                                    for j in range(2):
                                        kt_i = g * 2 + j
                                        lo = _lo(_r(g, j))
                                        if sc_mode == "fp8":
                                            nc.tensor.matmul(
                                                sc[:, j, lo:],
                                                qk_ap(h, kT8, kT8b)[
                                                    :, :, kt_i * 128:(kt_i + 1) * 128],
                                                qk_ap(h, qT8, qT8b)[
                                                    :, :, qi * QB + lo:(qi + 1) * QB],
                                                start=True, stop=True,
                                                perf_mode=DR,
                                            )
                                        else:
                                            nc.tensor.matmul(
                                                sc[:, j, lo:],
                                                kT8[off:off + 64, hv,
                                                    kt_i * 128:(kt_i + 1) * 128],
                                                qT8[off:off + 64, hv,
                                                    qi * QB + lo:(qi + 1) * QB],
                                                start=True, stop=True,
                                            )
                                    at = attp.tile([128, 2, QB], BF16, tag="at")
                                    lo01 = _lo(_r(g, 0))
                                    nc.scalar.activation(
                                        at[:, :, lo01:], sc[:, :, lo01:],
                                        mybir.ActivationFunctionType.Exp)
                                    for j in range(2):
                                        r = _r(g, j)
                                        if r is None:
                                            continue
                                        nc.vector.tensor_mul(
                                            at[:, j, r:r + 128],
                                            at[:, j, r:r + 128],
                                            masks[:, 0:128])
                                    att_tiles[h, g] = at
                                if g >= 1:
                                    ap = att_tiles.pop((h, g - 1))
                                    for j in range(2):
                                        kt_i = (g - 1) * 2 + j
                                        lo = _lo(_r(g - 1, j))
                                        nc.tensor.matmul(
                                            zts[h][0:HD + 1, lo:],
                                            vext[:, kt_i, h, :],
                                            ap[:, j, lo:],
                                            start=(g - 1 == 0 and j == 0),
                                            stop=(g - 1 == G - 1 and j == 1),
                                        )
                                if g == G:
                                    zt = zts[h]
                                    scr = smp.tile([128, QB], F32, tag="scr")
                                    bc = smp.tile([128, QB], F32, tag="bc")
                                    nc.vector.reciprocal(scr[0:1, :], zt[HD:HD + 1, :])
                                    nc.gpsimd.partition_broadcast(
                                        bc[:], scr[0:1, :], channels=128)
                                    nc.vector.tensor_mul(
                                        zTn[off:off + 64, hv, :],
                                        zt[0:HD, :], bc[off:off + 64, :])
                                step += 1
                                fill_maybe()
                while emitted < len(fill):
                    fill[emitted]()
                    emitted += 1
                return zTn

            def body(_i):
                load_inputs()
                with nc.named_scope("proj_q0"):
                    for c in projq_chunks(0) + projv_chunks(0):
                        c()
                # fill schedule: ACT-vs-PE deficit grows with the block index
                # (2.6us * (qi+1)), so defer the out-projections as late as
                # zTn buffering allows.
                zTn_prev = {}
                for qi in range(NQB):
                    fill = []
                    if qi + 1 < NQB:
                        with nc.named_scope(f"defer_proj_q{qi + 1}"):
                            fill += projq_chunks(qi + 1) + projv_chunks(qi + 1)
                    if qi == 3:
                        for qo in (0, 1, 2):
                            fill += oproj_chunks(qo, zTn_prev.pop(qo))
                    zTn_prev[qi] = attention(qi, fill)
                with nc.named_scope("oproj_q3"):
                    for c in oproj_final(NQB - 1, zTn_prev.pop(NQB - 1)):
                        c()

            if reps == 1:
                body(0)
            else:
                with tc.For_i(0, reps, 1, staggered_reset=True,
                              hint_engines=(mybir.EngineType.PE,)) as i:
                    body(i)

    nc.compile()
    return nc


def make_in_maps(X, W_qkv, W_out, sc_mode=None):
    """Host-side sharding: per-core input dict (bf16 DRAM traffic)."""
    import ml_dtypes
    bf16 = ml_dtypes.bfloat16
    sc_mode = sc_mode or SC_MODE

    X = np.asarray(X, dtype=np.float32)
    W_qkv = np.asarray(W_qkv, dtype=np.float32)
    W_out = np.asarray(W_out, dtype=np.float32)

    kp = np.arange(KT)[:, None]
    qf = np.arange(128)[None, :]
    masks = (qf >= kp).astype(bf16)

    if sc_mode == "fp8":
        # DoubleRow fold: PSUM partition p of jj-block holds local column
        # (p//32)*64 + jj*32 + (p%32)
        perm = np.array([(p // 32) * 64 + jj * 32 + (p % 32)
                         for jj in range(2) for p in range(128)])
    else:
        perm = np.arange(2 * DLOC // 2)    # identity over the 256 q/k cols

    in_maps = []
    for c in range(NCORES):
        b, hg = divmod(c, NGROUP)
        rows = slice(hg * DLOC, (hg + 1) * DLOC)
        wq = (W_qkv[0 * D:1 * D][rows] * 0.125)[perm].T  # fold 1/sqrt(hd) into q
        wk = W_qkv[1 * D:2 * D][rows][perm].T
        wv = W_qkv[2 * D:3 * D][rows].T
        in_maps.append({
            "xt": np.ascontiguousarray(X[b].T).astype(bf16),
            "wqkv": np.ascontiguousarray(
                np.concatenate([wq, wk, wv], axis=1)).astype(bf16),
            "wout": np.ascontiguousarray(W_out[:, rows].T).astype(bf16),
            "masks": masks,
        })
    return in_maps


def combine_outputs(results):
    """Sum the 4 tensor-parallel partials per batch -> [B, T, D]."""
    out = np.zeros((B, T, D), dtype=np.float32)
    for c, r in enumerate(results):
        out[c // NGROUP] += np.asarray(r["out"], dtype=np.float32)
    return out


_cached = {}


def kernel(X, W_qkv, W_out):
    from concourse.bass_utils import run_bass_kernel_spmd

    if "nc" not in _cached:
        _cached["nc"] = build_program(reps=1)
    nc = _cached["nc"]
    in_maps = make_in_maps(X, W_qkv, W_out, sc_mode=SC_MODE)
    r = run_bass_kernel_spmd(nc, in_maps, core_ids=list(range(NCORES)))
    return combine_outputs(r.results)
